# revision 48
# baseline (speedup 1.0000x reference)
"""Trainium2 Bass kernel for nn_LlamaApproximatedAttention.

Math (per batch b, with hs = hidden_states[b] [S, H]):
    F_h = W_seq @ hs            # [R, H]   (contract s)
    F_s = hs @ W_hid.T          # [S, R]   (contract h)
    out = F_s @ F_h             # [S, H]   (contract r)

Sharding: 8 cores = (batch b = c//2, seq-half j = c%2). Each core receives the
full hs[b] (own half + peer half), computes F_h fully, and F_s / out only for
its own half. Pure SPMD, no collectives (a pairwise AllReduce of F_h would
halve load DMA + e1 but costs ~20us latency).

Empirical cost model this schedule is built around (HW-measured micro):
  - matmul moving operand streams ~1 col/cycle at 2.4 GHz when both operands
    span 128 partitions; HALF rate when they span only 64 (rank-64 contraction)
  - 64-col stationaries (e1/e2) with rotating PSUM banks stream ~2x
  - batched PE transposes with the identity kept loaded: ~58 ns per 128x128
  - interleaving different stationaries (identity/weights) costs ~0.5us/swap
  - PSUM->SBUF copies (DVE/Act only) cost ~0.4us drain + ~0.5us/128Kelem
  - PE idle gaps re-throttle the clock to 1.2 GHz (HAM) -> keep the PE dense

Key math trick: einsum2 keeps its two half-contractions (h-tiles 0..7 / 8..15)
as SEPARATE partials stacked on partitions [F_sA; F_sB] [128, own-S], and
einsum3 contracts all 128 partitions against [F_h; F_h] (fh duplicated into
both halves): F_sA.T@F_h + F_sB.T@F_h == own out rows. That turns the rank-64
einsum3 into a full-width K=128 matmul (2x PE throughput), for free.

Key schedule trick: einsum3 is software-pipelined ACROSS bodies -- each body
starts with e3 of the PREVIOUS body's factors (operands long ready, stores
issue at the start of the DMA window), then runs TR -> e1 -> e2 for its own
data. An epilogue e3 after the loop finishes the last body. Factor tiles live
in an explicit 2-slot ring so the cross-body reference works inside a
hardware For loop.

DMA per body: 4 MiB own-half load + 4 x 1 MiB peer chunk loads (SP HWDGE
queue), 8 x 0.5 MiB stores (Pool SWDGE queue), each store from its own
staging tile so nothing waits on store completions. ~12.6 MiB of HBM traffic
-> ~38.5us floor at the measured ~327 GB/s/core.

grid_chw is unused by the math (it enumerates the full (s, h) grid).
"""

import numpy as np

import concourse.bass as bass  # noqa: F401  (engine namespaces hang off nc)
import concourse.mybir as mybir
import concourse.tile as tile
from concourse import bacc
from concourse.bass_utils import run_bass_kernel_spmd

B, S, H, R = 4, 2048, 2048, 64
N_CORES = 8
P = 128
T = S // P            # 16 s-tiles (also 16 h-tiles)
OWN_T = T // 2        # 8 own s-tiles per core
CHUNK = 512
NCH = H // CHUNK      # 4 h-chunks

f16 = mybir.dt.bfloat16
f32 = mybir.dt.float32
np_f16 = mybir.dt.np(f16)


def build_nc(reps: int = 1, mode: str = "full", unroll: bool = False):
    """Build + bacc-compile the SPMD kernel. reps>1 wraps copies of the body
    in a hardware For loop. Output is idempotent.

    mode (timing diagnostics, progressively enables phases):
      "dma"/"e1"/"tr"/"e2"/"full"; suffixes "-noload", "-nostore",
      "-nocopy3" (skip e3 cast copies+stores), "-conste3" (e3 reads const
      tiles), "-only3" (skip TR/e1/e2).
    """
    skip_loads = "-noload" in mode
    skip_stores = "-nostore" in mode
    skip_e3_copies = "-nocopy3" in mode
    const_e3 = "-conste3" in mode
    only_e3 = "-only3" in mode
    mode = mode.split("-")[0]
    lvl = {"dma": 0, "e1": 1, "tr": 2, "e2": 3, "full": 4}[mode]
    nc = bacc.Bacc(
        "TRN2",
        target_bir_lowering=False,
        debug=False,
        enable_asserts=True,
        num_devices=N_CORES,
    )

    # own half: one 4 MiB load [128, 8*H]; peer half: two 2 MiB chunk loads
    hso = nc.dram_tensor("hso", [P, 8 * H], f16, kind="ExternalInput").ap()
    hsp = nc.dram_tensor("hsp", [2, P, 4 * H], f16, kind="ExternalInput").ap()
    wst = nc.dram_tensor("wst", [P, T * R], f16, kind="ExternalInput").ap()
    wht = nc.dram_tensor("wht", [P, T * R], f16, kind="ExternalInput").ap()
    ident = nc.dram_tensor("ident", [P, P], f16, kind="ExternalInput").ap()
    # store j holds own s-tiles (j, j+4): [128, 2*H] bf16 (1 MiB)
    out = nc.dram_tensor("out", [OWN_T // 2, P, 2 * H], f16,
                         kind="ExternalOutput").ap()

    with tile.TileContext(nc) as tc:
        with (
            tc.tile_pool(name="const", bufs=1) as cpool,
            tc.tile_pool(name="raw", bufs=2) as rpool,
            tc.tile_pool(name="peer", bufs=1) as ppool,
            tc.tile_pool(name="hsT", bufs=1) as hpool,
            tc.tile_pool(name="fact", bufs=1) as fpool,
            tc.tile_pool(name="outsb", bufs=1) as opool,
        ):
            # constants: loaded once, outside the timing loop
            wst_t = cpool.tile([P, T * R], f16, tag="wst")
            wht_t = cpool.tile([P, T * R], f16, tag="wht")
            id_t = cpool.tile([P, P], f16, tag="id")
            nc.gpsimd.dma_start(wst_t[:], wst)
            nc.gpsimd.dma_start(wht_t[:], wht)
            nc.gpsimd.dma_start(id_t[:], ident)
            fake_out = None
            if lvl < 4:
                fake_out = cpool.tile([P, 2 * H], f16, tag="fake_out")
                nc.gpsimd.memset(fake_out[:], 0.0)
            cfsta = cfstb = cfh = None
            if const_e3:
                cfsta = cpool.tile([P, 4 * P], f16, tag="cfsta")
                cfstb = cpool.tile([P, 4 * P], f16, tag="cfstb")
                cfh = cpool.tile([P, H], f16, tag="cfh")
                nc.vector.memset(cfsta[:], 0.5)
                nc.vector.memset(cfstb[:], 0.5)
                nc.vector.memset(cfh[:], 0.5)

            # explicit 2-slot ring of factor tiles (cross-body e3 pipelining)
            slots = []
            for sl in range(2):
                slots.append({
                    "fsta": fpool.tile([P, 4 * P], f16, tag=f"fsta{sl}",
                                       name=f"fsta{sl}"),
                    "fstb": fpool.tile([P, 4 * P], f16, tag=f"fstb{sl}",
                                       name=f"fstb{sl}"),
                    "fh": fpool.tile([P, H], f16, tag=f"fh{sl}",
                                     name=f"fh{sl}"),
                })

            # alternate PSUM->SBUF cast copies between DVE and Act
            cp_state = {"n": 0}

            def cast_copy(dst, src):
                if cp_state["n"] % 2 == 0:
                    nc.vector.tensor_copy(dst, src)
                else:
                    nc.scalar.copy(dst, src)
                cp_state["n"] += 1

            def e3(slot):
                """einsum3 of a body's factors + stores; K=128 full width.
                Own s-tiles visited 0,4,1,5,... so the stationary alternates
                between the fsta/fstb tiles (lets LDW pull ahead)."""
                if lvl < 4:
                    return
                if const_e3:
                    fa, fb, fh = cfsta, cfstb, cfh
                else:
                    fa, fb, fh = slot["fsta"], slot["fstb"], slot["fh"]
                with tc.tile_pool(name="ps_o", bufs=4, space="PSUM") as po:
                    outsb = None
                    for ii in range(OWN_T):
                        tile_i = (ii // 2) * P
                        stat = fa if ii % 2 == 0 else fb
                        if ii % 2 == 0:
                            outsb = opool.tile([P, 2 * H], f16,
                                               tag=f"outsb{ii // 2}",
                                               name=f"outsb{ii // 2}")
                        for c0 in range(2):
                            ps_o = po.tile([P, 2 * CHUNK], f32, tag="o")
                            for cc in range(2):
                                nc.tensor.matmul(
                                    ps_o[:, cc * CHUNK:(cc + 1) * CHUNK],
                                    stat[:, tile_i:tile_i + P],
                                    fh[:, (2 * c0 + cc) * CHUNK:
                                       (2 * c0 + cc + 1) * CHUNK],
                                    start=True,
                                    stop=True,
                                )
                            if not skip_e3_copies:
                                cast_copy(
                                    outsb[:, (ii % 2) * H + c0 * 2 * CHUNK:
                                          (ii % 2) * H + (c0 + 1) * 2 * CHUNK],
                                    ps_o[:],
                                )
                        if ii % 2 == 1 and not skip_stores \
                                and not skip_e3_copies:
                            nc.gpsimd.dma_start(out[ii // 2], outsb[:])

            def body(slot_idx, prev_has_data):
                slot = slots[slot_idx]

                own = rpool.tile([P, 8 * H], f16, tag="own")
                if not skip_loads:
                    nc.sync.dma_start(own[:], hso)
                else:
                    nc.sync.dma_start(own[:, 0:16], hso[:, 0:16])
                peers = []
                for pc in range(2):
                    # bufs=1: the next body's peer load WAW-waits this body's
                    # e1 peer reads, which finish mid-body -- safe
                    pt = ppool.tile([P, 4 * H], f16, tag=f"peer{pc}",
                                    name=f"peer{pc}")
                    if not skip_loads:
                        nc.sync.dma_start(pt[:], hsp[pc])
                    else:
                        nc.sync.dma_start(pt[:, 0:16], hsp[pc, :, 0:16])
                    peers.append(pt)

                # e3 of the PREVIOUS body's factors first: operands are long
                # ready (no entry latency), stores hit the DMA queue at the
                # start of this body's window
                if prev_has_data or const_e3:
                    e3(slots[1 - slot_idx])

                if only_e3:
                    return

                def s_tile_ap(t, k):
                    """[128, 128] slice of s-tile t, h-tile k."""
                    if t < OWN_T:
                        return own[:, t * H + k * P:t * H + (k + 1) * P]
                    pt = peers[(t - OWN_T) // 4]
                    off = ((t - OWN_T) % 4) * H
                    return pt[:, off + k * P:off + (k + 1) * P]

                hsTs = {}

                def tr_all(trp):
                    # transpose own 8 s-tiles for all 16 h-tiles (identity
                    # stays loaded); h-tile PAIRS share one PSUM tile and one
                    # wide [128, 2048] copy (halves the per-copy drain tax)
                    if lvl < 2:
                        return
                    for m in range(T // 2):
                        ps_tr = trp.tile([P, T * P], f16, tag="tr")
                        for k2 in range(2):
                            k = 2 * m + k2
                            for q in range(OWN_T):
                                nc.tensor.matmul(
                                    ps_tr[:, (k2 * OWN_T + q) * P:
                                          (k2 * OWN_T + q + 1) * P],
                                    s_tile_ap(q, k),
                                    id_t[:],
                                    is_transpose=True,
                                    start=(k2 == 0 and q == 0),
                                    stop=(k2 == 1 and q == OWN_T - 1),
                                )
                        hsT = hpool.tile([P, T * P], f16, tag=f"hsT{m}")
                        cast_copy(hsT[:], ps_tr[:])
                        hsTs[m] = hsT

                def e1(g, ps_fh):
                    # einsum1: accumulate partial F_h from s-group g
                    def fh_ps(c):
                        return ps_fh[(c // 2) * R:(c // 2 + 1) * R,
                                     (c % 2) * CHUNK:(c % 2 + 1) * CHUNK]

                    for q in range(4 if lvl >= 1 else 0):
                        t = 4 * g + q
                        if t < OWN_T:
                            src, base = own, t * H
                        else:
                            src = peers[(t - OWN_T) // 4]
                            base = ((t - OWN_T) % 4) * H
                        for c in range(NCH):
                            nc.tensor.matmul(
                                fh_ps(c),
                                wst_t[:, t * R:(t + 1) * R],
                                src[:, base + c * CHUNK:base + (c + 1) * CHUNK],
                                start=(t == 0),
                                stop=(t == T - 1),
                            )

                def fh_copies(ps_fh):
                    # F_h -> SBUF, duplicated to both partition halves;
                    # drains on DVE/Act during e2's matmul stream
                    if lvl < 4:
                        return
                    for ph in range(2):
                        src = ps_fh[ph * R:(ph + 1) * R, :]
                        sl = slice(ph * 2 * CHUNK, (ph + 1) * 2 * CHUNK)
                        nc.vector.tensor_copy(slot["fh"][0:R, sl], src)
                        nc.scalar.copy(slot["fh"][R:2 * R, sl], src)

                def e2_all(fs_psp):
                    # einsum2, both h-halves interleaved: partial A (h-tiles
                    # 0..7) on partitions 0:64 / cols 0:1024, partial B
                    # (8..15) on partitions 64:128 / cols 1024:2048 ->
                    # consecutive matmuls rotate over 4 PSUM banks
                    if lvl < 3:
                        return
                    ps_fs = fs_psp.tile([P, 4 * CHUNK], f32, tag="fs",
                                        name="ps_fs")
                    for j in range(8):
                        for cc in range(2):
                            for hp in range(2):
                                k = hp * 8 + j
                                nc.tensor.matmul(
                                    ps_fs[hp * R:(hp + 1) * R,
                                          hp * 2 * CHUNK + cc * CHUNK:
                                          hp * 2 * CHUNK + (cc + 1) * CHUNK],
                                    wht_t[:, k * R:(k + 1) * R],
                                    hsTs[k // 2][:, (k % 2) * OWN_T * P +
                                                 cc * CHUNK:
                                                 (k % 2) * OWN_T * P +
                                                 (cc + 1) * CHUNK],
                                    start=(j == 0),
                                    stop=(j == 7),
                                )
                    # F_s.T partials -> SBUF: a = own s-tiles 0..3, b = 4..7
                    cast_copy(slot["fsta"][0:R, :], ps_fs[0:R, 0:4 * P])
                    cast_copy(slot["fstb"][0:R, :], ps_fs[0:R, 4 * P:8 * P])
                    cast_copy(slot["fsta"][R:P, :],
                              ps_fs[R:P, 2 * CHUNK:2 * CHUNK + 4 * P])
                    cast_copy(slot["fstb"][R:P, :],
                              ps_fs[R:P, 2 * CHUNK + 4 * P:4 * CHUNK])

                with tc.tile_pool(name="ps_tr", bufs=3, space="PSUM") as trp:
                    tr_all(trp)
                with tc.tile_pool(name="ps_fh", bufs=1, space="PSUM") as fh_psp:
                    ps_fh = fh_psp.tile([P, 2 * CHUNK], f32, tag="fh",
                                        name="ps_fh")
                    e1(0, ps_fh)
                    e1(1, ps_fh)
                    e1(2, ps_fh)
                    e1(3, ps_fh)
                    fh_copies(ps_fh)
                    with tc.tile_pool(name="ps_fs", bufs=1,
                                      space="PSUM") as fs_psp:
                        e2_all(fs_psp)

                if lvl < 4 and not skip_stores:
                    for sp in range(OWN_T // 2):
                        nc.gpsimd.dma_start(out[sp], fake_out[:])

            if unroll or reps == 1:
                for i in range(reps):
                    body(i % 2, i > 0)
                e3(slots[(reps - 1) % 2])
            else:
                nbody = 8 if reps % 8 == 0 else (4 if reps % 4 == 0 else 2)
                assert reps % nbody == 0
                with tc.For_i(0, reps // nbody, 1):
                    for i in range(nbody):
                        # steady state: every body's e3 reads the other slot
                        # (the previous body's factors; first iteration's
                        # body0 reads garbage but out is rewritten each
                        # iteration -> final state correct)
                        body(i % 2, True)
                e3(slots[(nbody - 1) % 2])

    nc.compile()
    return nc


def _tile_weight(w_t: np.ndarray) -> np.ndarray:
    """[2048, 64] -> [128, 16*64] stack where tile t = cols [64t:64t+64]."""
    return np.ascontiguousarray(
        w_t.reshape(T, P, R).transpose(1, 0, 2).reshape(P, T * R)
    ).astype(np_f16)


def _pack_half(hs_h: np.ndarray) -> np.ndarray:
    """[1024, 2048] -> [128, 8*H]: row p holds s-tile rows 128*t + p
    concatenated over the half's 8 s-tiles t."""
    return np.ascontiguousarray(
        hs_h.reshape(8, P, H).transpose(1, 0, 2).reshape(P, 8 * H)
    )


def _unpack_out(o: np.ndarray) -> np.ndarray:
    """[4, 128, 2*H] -> [1024, 2048]: store j half w holds s-tile 4w + j."""
    return np.ascontiguousarray(
        o.reshape(OWN_T // 2, P, 2, H).transpose(2, 0, 1, 3).reshape(S // 2, H)
    )


def build_in_maps(hs_all: np.ndarray, w_seq: np.ndarray, w_hid: np.ndarray):
    ident = np.eye(P, dtype=np_f16)
    wht_tiled = _tile_weight(np.ascontiguousarray(w_hid.T))
    hs_f16 = hs_all.astype(np_f16)
    wst_halves = {
        j: _tile_weight(np.roll(w_seq.T, -(S // 2) * j, axis=0))
        for j in range(2)
    }
    in_maps = []
    for c in range(N_CORES):
        b, j = c // 2, c % 2
        hsb = hs_f16[b]
        own_h = hsb[j * (S // 2):(j + 1) * (S // 2)]
        peer_h = hsb[(1 - j) * (S // 2):(2 - j) * (S // 2)]
        in_maps.append(
            {"hso": _pack_half(own_h),
             "hsp": _pack_half(peer_h).reshape(P, 2, 4 * H)
                    .transpose(1, 0, 2).copy(),
             "wst": wst_halves[j], "wht": wht_tiled, "ident": ident}
        )
    return in_maps


_NC_CACHE: dict = {}


def kernel(**inputs) -> np.ndarray:
    hs_all = np.asarray(inputs["hidden_states"], dtype=np.float32)
    w_seq = np.asarray(inputs["W_seq"], dtype=np.float32)
    w_hid = np.asarray(inputs["W_hid"], dtype=np.float32)

    if "nc" not in _NC_CACHE:
        _NC_CACHE["nc"] = build_nc(1)
    nc = _NC_CACHE["nc"]

    in_maps = build_in_maps(hs_all, w_seq, w_hid)
    res = run_bass_kernel_spmd(nc, in_maps, core_ids=list(range(N_CORES)))

    out_full = np.empty((B, S, H), dtype=np.float32)
    for c in range(N_CORES):
        b, j = c // 2, c % 2
        out_full[b, j * (S // 2):(j + 1) * (S // 2), :] = _unpack_out(
            res.results[c]["out"]
        )
    return out_full


# revision 51
# speedup vs baseline: 1.0025x; 1.0025x over previous
"""Trainium2 Bass kernel for nn_LlamaApproximatedAttention.

Math (per batch b, with hs = hidden_states[b] [S, H]):
    F_h = W_seq @ hs            # [R, H]   (contract s)
    F_s = hs @ W_hid.T          # [S, R]   (contract h)
    out = F_s @ F_h             # [S, H]   (contract r)

Sharding: 8 cores = (batch b = c//2, seq-half j = c%2). Each core receives the
full hs[b] (own half + peer half), computes F_h fully, and F_s / out only for
its own half. Pure SPMD, no collectives (a pairwise AllReduce of F_h would
halve load DMA + e1 but costs ~20us latency).

Empirical cost model this schedule is built around (HW-measured micro):
  - matmul moving operand streams ~1 col/cycle at 2.4 GHz when both operands
    span 128 partitions; HALF rate when they span only 64 (rank-64 contraction)
  - 64-col stationaries (e1/e2) with rotating PSUM banks stream ~2x
  - batched PE transposes with the identity kept loaded: ~58 ns per 128x128
  - interleaving different stationaries (identity/weights) costs ~0.5us/swap
  - PSUM->SBUF copies (DVE/Act only) cost ~0.4us drain + ~0.5us/128Kelem
  - PE idle gaps re-throttle the clock to 1.2 GHz (HAM) -> keep the PE dense

Key math trick: einsum2 keeps its two half-contractions (h-tiles 0..7 / 8..15)
as SEPARATE partials stacked on partitions [F_sA; F_sB] [128, own-S], and
einsum3 contracts all 128 partitions against [F_h; F_h] (fh duplicated into
both halves): F_sA.T@F_h + F_sB.T@F_h == own out rows. That turns the rank-64
einsum3 into a full-width K=128 matmul (2x PE throughput), for free.

Key schedule trick: einsum3 is software-pipelined ACROSS bodies -- each body
starts with e3 of the PREVIOUS body's factors (operands long ready, stores
issue at the start of the DMA window), then runs TR -> e1 -> e2 for its own
data. An epilogue e3 after the loop finishes the last body. Factor tiles live
in an explicit 2-slot ring so the cross-body reference works inside a
hardware For loop.

DMA per body: 4 MiB own-half load + 2 x 2 MiB peer chunk loads (SP HWDGE
queue), 4 x 1 MiB stores (Pool SWDGE queue), each store from its own staging
tile so nothing waits on store completions. ~12.6 MiB of HBM traffic
-> ~38.5us floor at the measured ~327 GB/s/core (fp8 would cut bytes but
fails the 2e-2 error budget; a pairwise collective cannot sit inside the
timing For loop).

grid_chw is unused by the math (it enumerates the full (s, h) grid).
"""

import numpy as np

import concourse.bass as bass  # noqa: F401  (engine namespaces hang off nc)
import concourse.mybir as mybir
import concourse.tile as tile
from concourse import bacc
from concourse.bass_utils import run_bass_kernel_spmd

B, S, H, R = 4, 2048, 2048, 64
N_CORES = 8
P = 128
T = S // P            # 16 s-tiles (also 16 h-tiles)
OWN_T = T // 2        # 8 own s-tiles per core
CHUNK = 512
NCH = H // CHUNK      # 4 h-chunks

f16 = mybir.dt.bfloat16
f32 = mybir.dt.float32
np_f16 = mybir.dt.np(f16)


def build_nc(reps: int = 1, mode: str = "full", unroll: bool = False):
    """Build + bacc-compile the SPMD kernel. reps>1 wraps copies of the body
    in a hardware For loop. Output is idempotent.

    mode (timing diagnostics, progressively enables phases):
      "dma"/"e1"/"tr"/"e2"/"full"; suffixes "-noload", "-nostore",
      "-nocopy3" (skip e3 cast copies+stores), "-conste3" (e3 reads const
      tiles), "-only3" (skip TR/e1/e2).
    """
    skip_loads = "-noload" in mode
    skip_stores = "-nostore" in mode
    skip_e3_copies = "-nocopy3" in mode
    const_e3 = "-conste3" in mode
    only_e3 = "-only3" in mode
    mode = mode.split("-")[0]
    lvl = {"dma": 0, "e1": 1, "tr": 2, "e2": 3, "full": 4}[mode]
    nc = bacc.Bacc(
        "TRN2",
        target_bir_lowering=False,
        debug=False,
        enable_asserts=True,
        num_devices=N_CORES,
    )

    # own half: one 4 MiB load [128, 8*H]; peer half: two 2 MiB chunk loads
    # (footprint small enough to single-buffer, freeing SBUF for staging)
    hso = nc.dram_tensor("hso", [P, 8 * H], f16, kind="ExternalInput").ap()
    hsp = nc.dram_tensor("hsp", [2, P, 4 * H], f16, kind="ExternalInput").ap()
    wst = nc.dram_tensor("wst", [P, T * R], f16, kind="ExternalInput").ap()
    wht = nc.dram_tensor("wht", [P, T * R], f16, kind="ExternalInput").ap()
    ident = nc.dram_tensor("ident", [P, P], f16, kind="ExternalInput").ap()
    # store j holds own s-tiles (j, j+4): [128, 2*H] bf16 (1 MiB)
    out = nc.dram_tensor("out", [OWN_T // 2, P, 2 * H], f16,
                         kind="ExternalOutput").ap()

    with tile.TileContext(nc) as tc:
        with (
            tc.tile_pool(name="const", bufs=1) as cpool,
            tc.tile_pool(name="raw", bufs=2) as rpool,
            tc.tile_pool(name="peer", bufs=1) as ppool,
            tc.tile_pool(name="hsT", bufs=1) as hpool,
            tc.tile_pool(name="fact", bufs=1) as fpool,
            tc.tile_pool(name="outsb", bufs=1) as opool,
        ):
            # constants: loaded once, outside the timing loop
            wst_t = cpool.tile([P, T * R], f16, tag="wst")
            wht_t = cpool.tile([P, T * R], f16, tag="wht")
            id_t = cpool.tile([P, P], f16, tag="id")
            nc.gpsimd.dma_start(wst_t[:], wst)
            nc.gpsimd.dma_start(wht_t[:], wht)
            nc.gpsimd.dma_start(id_t[:], ident)
            fake_out = None
            if lvl < 4:
                fake_out = cpool.tile([P, 2 * H], f16, tag="fake_out")
                nc.gpsimd.memset(fake_out[:], 0.0)
            cfsta = cfstb = cfh = None
            if const_e3:
                cfsta = cpool.tile([P, 4 * P], f16, tag="cfsta")
                cfstb = cpool.tile([P, 4 * P], f16, tag="cfstb")
                cfh = cpool.tile([P, H], f16, tag="cfh")
                nc.vector.memset(cfsta[:], 0.5)
                nc.vector.memset(cfstb[:], 0.5)
                nc.vector.memset(cfh[:], 0.5)

            # explicit 2-slot ring of factor tiles (cross-body e3 pipelining)
            slots = []
            for sl in range(2):
                slots.append({
                    "fsta": fpool.tile([P, 4 * P], f16, tag=f"fsta{sl}",
                                       name=f"fsta{sl}"),
                    "fstb": fpool.tile([P, 4 * P], f16, tag=f"fstb{sl}",
                                       name=f"fstb{sl}"),
                    "fh": fpool.tile([P, H], f16, tag=f"fh{sl}",
                                     name=f"fh{sl}"),
                })

            # alternate PSUM->SBUF cast copies between DVE and Act
            cp_state = {"n": 0}

            def cast_copy(dst, src):
                if cp_state["n"] % 2 == 0:
                    nc.vector.tensor_copy(dst, src)
                else:
                    nc.scalar.copy(dst, src)
                cp_state["n"] += 1

            def e3(slot):
                """einsum3 of a body's factors + stores; K=128 full width.
                Own s-tiles visited 0,4,1,5,... so the stationary alternates
                between the fsta/fstb tiles (lets LDW pull ahead)."""
                if lvl < 4:
                    return
                if const_e3:
                    fa, fb, fh = cfsta, cfstb, cfh
                else:
                    fa, fb, fh = slot["fsta"], slot["fstb"], slot["fh"]
                with tc.tile_pool(name="ps_o", bufs=4, space="PSUM") as po:
                    outsb = None
                    for ii in range(OWN_T):
                        tile_i = (ii // 2) * P
                        stat = fa if ii % 2 == 0 else fb
                        if ii % 2 == 0:
                            outsb = opool.tile([P, 2 * H], f16,
                                               tag=f"outsb{ii // 2}",
                                               name=f"outsb{ii // 2}")
                        for c0 in range(2):
                            ps_o = po.tile([P, 2 * CHUNK], f32, tag="o")
                            for cc in range(2):
                                nc.tensor.matmul(
                                    ps_o[:, cc * CHUNK:(cc + 1) * CHUNK],
                                    stat[:, tile_i:tile_i + P],
                                    fh[:, (2 * c0 + cc) * CHUNK:
                                       (2 * c0 + cc + 1) * CHUNK],
                                    start=True,
                                    stop=True,
                                )
                            if not skip_e3_copies:
                                cast_copy(
                                    outsb[:, (ii % 2) * H + c0 * 2 * CHUNK:
                                          (ii % 2) * H + (c0 + 1) * 2 * CHUNK],
                                    ps_o[:],
                                )
                        if ii % 2 == 1 and not skip_stores \
                                and not skip_e3_copies:
                            nc.gpsimd.dma_start(out[ii // 2], outsb[:])

            def body(slot_idx, prev_has_data):
                slot = slots[slot_idx]

                own = rpool.tile([P, 8 * H], f16, tag="own")
                if not skip_loads:
                    nc.sync.dma_start(own[:], hso)
                else:
                    nc.sync.dma_start(own[:, 0:16], hso[:, 0:16])
                peers = []
                for pc in range(2):
                    # bufs=1: the next body's peer load WAW-waits this body's
                    # e1 peer reads, which finish mid-body -- safe
                    pt = ppool.tile([P, 4 * H], f16, tag=f"peer{pc}",
                                    name=f"peer{pc}")
                    if not skip_loads:
                        nc.sync.dma_start(pt[:], hsp[pc])
                    else:
                        nc.sync.dma_start(pt[:, 0:16], hsp[pc, :, 0:16])
                    peers.append(pt)

                # e3 of the PREVIOUS body's factors first: operands are long
                # ready (no entry latency), stores hit the DMA queue at the
                # start of this body's window
                if prev_has_data or const_e3:
                    e3(slots[1 - slot_idx])

                if only_e3:
                    return

                def s_tile_ap(t, k):
                    """[128, 128] slice of s-tile t, h-tile k."""
                    if t < OWN_T:
                        return own[:, t * H + k * P:t * H + (k + 1) * P]
                    pt = peers[(t - OWN_T) // 4]
                    off = ((t - OWN_T) % 4) * H
                    return pt[:, off + k * P:off + (k + 1) * P]

                hsTs = {}

                def tr_all(trp):
                    # transpose own 8 s-tiles for all 16 h-tiles (identity
                    # stays loaded); h-tile PAIRS share one PSUM tile and one
                    # wide [128, 2048] copy (halves the per-copy drain tax)
                    if lvl < 2:
                        return
                    for m in range(T // 2):
                        ps_tr = trp.tile([P, T * P], f16, tag="tr")
                        for k2 in range(2):
                            k = 2 * m + k2
                            for q in range(OWN_T):
                                nc.tensor.matmul(
                                    ps_tr[:, (k2 * OWN_T + q) * P:
                                          (k2 * OWN_T + q + 1) * P],
                                    s_tile_ap(q, k),
                                    id_t[:],
                                    is_transpose=True,
                                    start=(k2 == 0 and q == 0),
                                    stop=(k2 == 1 and q == OWN_T - 1),
                                )
                        hsT = hpool.tile([P, T * P], f16, tag=f"hsT{m}")
                        cast_copy(hsT[:], ps_tr[:])
                        hsTs[m] = hsT

                def e1(g, ps_fh):
                    # einsum1: accumulate partial F_h from s-group g
                    def fh_ps(c):
                        return ps_fh[(c // 2) * R:(c // 2 + 1) * R,
                                     (c % 2) * CHUNK:(c % 2 + 1) * CHUNK]

                    for q in range(4 if lvl >= 1 else 0):
                        t = 4 * g + q
                        if t < OWN_T:
                            src, base = own, t * H
                        else:
                            src = peers[(t - OWN_T) // 4]
                            base = ((t - OWN_T) % 4) * H
                        for c in range(NCH):
                            nc.tensor.matmul(
                                fh_ps(c),
                                wst_t[:, t * R:(t + 1) * R],
                                src[:, base + c * CHUNK:base + (c + 1) * CHUNK],
                                start=(t == 0),
                                stop=(t == T - 1),
                            )

                def fh_copies(ps_fh):
                    # F_h -> SBUF, duplicated to both partition halves;
                    # drains on DVE/Act during e2's matmul stream
                    if lvl < 4:
                        return
                    for ph in range(2):
                        src = ps_fh[ph * R:(ph + 1) * R, :]
                        sl = slice(ph * 2 * CHUNK, (ph + 1) * 2 * CHUNK)
                        nc.vector.tensor_copy(slot["fh"][0:R, sl], src)
                        nc.scalar.copy(slot["fh"][R:2 * R, sl], src)

                def e2_all(fs_psp):
                    # einsum2, both h-halves interleaved: partial A (h-tiles
                    # 0..7) on partitions 0:64 / cols 0:1024, partial B
                    # (8..15) on partitions 64:128 / cols 1024:2048 ->
                    # consecutive matmuls rotate over 4 PSUM banks
                    if lvl < 3:
                        return
                    ps_fs = fs_psp.tile([P, 4 * CHUNK], f32, tag="fs",
                                        name="ps_fs")
                    for j in range(8):
                        for cc in range(2):
                            for hp in range(2):
                                k = hp * 8 + j
                                nc.tensor.matmul(
                                    ps_fs[hp * R:(hp + 1) * R,
                                          hp * 2 * CHUNK + cc * CHUNK:
                                          hp * 2 * CHUNK + (cc + 1) * CHUNK],
                                    wht_t[:, k * R:(k + 1) * R],
                                    hsTs[k // 2][:, (k % 2) * OWN_T * P +
                                                 cc * CHUNK:
                                                 (k % 2) * OWN_T * P +
                                                 (cc + 1) * CHUNK],
                                    start=(j == 0),
                                    stop=(j == 7),
                                )
                    # F_s.T partials -> SBUF: a = own s-tiles 0..3, b = 4..7
                    cast_copy(slot["fsta"][0:R, :], ps_fs[0:R, 0:4 * P])
                    cast_copy(slot["fstb"][0:R, :], ps_fs[0:R, 4 * P:8 * P])
                    cast_copy(slot["fsta"][R:P, :],
                              ps_fs[R:P, 2 * CHUNK:2 * CHUNK + 4 * P])
                    cast_copy(slot["fstb"][R:P, :],
                              ps_fs[R:P, 2 * CHUNK + 4 * P:4 * CHUNK])

                with tc.tile_pool(name="ps_tr", bufs=3, space="PSUM") as trp:
                    tr_all(trp)
                with tc.tile_pool(name="ps_fh", bufs=1, space="PSUM") as fh_psp:
                    ps_fh = fh_psp.tile([P, 2 * CHUNK], f32, tag="fh",
                                        name="ps_fh")
                    e1(0, ps_fh)
                    e1(1, ps_fh)
                    e1(2, ps_fh)
                    e1(3, ps_fh)
                    fh_copies(ps_fh)
                    with tc.tile_pool(name="ps_fs", bufs=1,
                                      space="PSUM") as fs_psp:
                        e2_all(fs_psp)

                if lvl < 4 and not skip_stores:
                    for sp in range(OWN_T // 2):
                        nc.gpsimd.dma_start(out[sp], fake_out[:])

            if unroll or reps == 1:
                for i in range(reps):
                    body(i % 2, i > 0)
                e3(slots[(reps - 1) % 2])
            else:
                nbody = 8 if reps % 8 == 0 else (4 if reps % 4 == 0 else 2)
                assert reps % nbody == 0
                with tc.For_i(0, reps // nbody, 1):
                    for i in range(nbody):
                        # steady state: every body's e3 reads the other slot
                        # (the previous body's factors; first iteration's
                        # body0 reads garbage but out is rewritten each
                        # iteration -> final state correct)
                        body(i % 2, True)
                e3(slots[(nbody - 1) % 2])

    nc.compile()
    return nc


def _tile_weight(w_t: np.ndarray) -> np.ndarray:
    """[2048, 64] -> [128, 16*64] stack where tile t = cols [64t:64t+64]."""
    return np.ascontiguousarray(
        w_t.reshape(T, P, R).transpose(1, 0, 2).reshape(P, T * R)
    ).astype(np_f16)


def _pack_half(hs_h: np.ndarray) -> np.ndarray:
    """[1024, 2048] -> [128, 8*H]: row p holds s-tile rows 128*t + p
    concatenated over the half's 8 s-tiles t."""
    return np.ascontiguousarray(
        hs_h.reshape(8, P, H).transpose(1, 0, 2).reshape(P, 8 * H)
    )


def _unpack_out(o: np.ndarray) -> np.ndarray:
    """[4, 128, 2*H] -> [1024, 2048]: store j half w holds s-tile 4w + j."""
    return np.ascontiguousarray(
        o.reshape(OWN_T // 2, P, 2, H).transpose(2, 0, 1, 3).reshape(S // 2, H)
    )


def build_in_maps(hs_all: np.ndarray, w_seq: np.ndarray, w_hid: np.ndarray):
    ident = np.eye(P, dtype=np_f16)
    wht_tiled = _tile_weight(np.ascontiguousarray(w_hid.T))
    hs_f16 = hs_all.astype(np_f16)
    wst_halves = {
        j: _tile_weight(np.roll(w_seq.T, -(S // 2) * j, axis=0))
        for j in range(2)
    }
    in_maps = []
    for c in range(N_CORES):
        b, j = c // 2, c % 2
        hsb = hs_f16[b]
        own_h = hsb[j * (S // 2):(j + 1) * (S // 2)]
        peer_h = hsb[(1 - j) * (S // 2):(2 - j) * (S // 2)]
        in_maps.append(
            {"hso": _pack_half(own_h),
             "hsp": _pack_half(peer_h).reshape(P, 2, 4 * H)
                    .transpose(1, 0, 2).copy(),
             "wst": wst_halves[j], "wht": wht_tiled, "ident": ident}
        )
    return in_maps


_NC_CACHE: dict = {}


def kernel(**inputs) -> np.ndarray:
    hs_all = np.asarray(inputs["hidden_states"], dtype=np.float32)
    w_seq = np.asarray(inputs["W_seq"], dtype=np.float32)
    w_hid = np.asarray(inputs["W_hid"], dtype=np.float32)

    if "nc" not in _NC_CACHE:
        _NC_CACHE["nc"] = build_nc(1)
    nc = _NC_CACHE["nc"]

    in_maps = build_in_maps(hs_all, w_seq, w_hid)
    res = run_bass_kernel_spmd(nc, in_maps, core_ids=list(range(N_CORES)))

    out_full = np.empty((B, S, H), dtype=np.float32)
    for c in range(N_CORES):
        b, j = c // 2, c % 2
        out_full[b, j * (S // 2):(j + 1) * (S // 2), :] = _unpack_out(
            res.results[c]["out"]
        )
    return out_full
